# revision 4
# baseline (speedup 1.0000x reference)
"""Trainium2 Bass kernel for nn_GTConv (gnn_message_passing).

Computes, for R=8 relation graphs with E=2M edges each and C=4 output
channels:
    Filter = softmax(weight, axis=1)                  # [C, R]
    out[c, r, e] = Filter[c, r] * edge_w[r, e]        # [C, R, E]

Sharding: edge dim E split across 8 NeuronCores (data parallel); the
tiny [C, R] weight is replicated.

Per-core layout: the [R=8, E/8=250000] edge shard is viewed as
[128, 15625] (flat-preserving reshape: partition p = r*16 + j covers
edges e = j*15625 + i of relation r = p//16), then stored chunk-major
([NCH, 128, CH]) so every chunk DMA is a single contiguous 1.6 MB run.
The per-partition scale Filter[c, p//16] is materialized on-device as an
SBUF tile fscal[128, C] via two tiny PE matmuls against a replication
mask and an all-ones mask, so each output channel is one per-partition
tensor_scalar multiply on DVE.

Engine split (keeps both HWDGE rings saturated and unblocked):
  - nc.sync   : the NCH input-chunk loads only (issued first, no deps)
  - nc.scalar : store issues (second HWDGE ring) + 2 tiny Exp ops
  - nc.vector : all channel multiplies
  - nc.gpsimd : tiny preamble DMAs (SWDGE, off the HWDGE rings)
"""

import numpy as np

import concourse.bacc as bacc
import concourse.mybir as mybir
import concourse.tile as tile
from concourse.bass_utils import run_bass_kernel_spmd

R = 8                 # relations (in_channels)
C = 4                 # out_channels
E = 2_000_000         # edges per relation
NCORES = 8
ES = E // NCORES      # 250_000 edges per core
P = 128               # SBUF partitions
JP = P // R           # 16 partition-groups per relation
F = ES // JP          # 15625 free elems per partition
CH = 3125             # free-dim chunk size
NCH = F // CH         # 5 chunks

FP32 = mybir.dt.float32

_cached = {}


def _build():
    nc = bacc.Bacc("TRN2", target_bir_lowering=False, debug=False,
                   num_devices=NCORES)
    ew = nc.dram_tensor("ew", [NCH, P, CH], FP32, kind="ExternalInput")
    w = nc.dram_tensor("w", [C, R], FP32, kind="ExternalInput")
    out = nc.dram_tensor("out", [C, NCH, P, CH], FP32, kind="ExternalOutput")
    filt = nc.dram_tensor("filt", [C, R], FP32, kind="ExternalOutput")
    # rep[r, p] = 1 iff p // JP == r (NEFF-embedded const)
    rep_np = np.repeat(np.eye(R, dtype=np.float32), JP, axis=1)
    rep_dram = nc.inline_tensor(rep_np, name="rep_const")

    with tile.TileContext(nc) as tc:
        with (
            tc.tile_pool(name="small", bufs=1) as small,
            tc.tile_pool(name="psum", bufs=1, space="PSUM") as psum,
            tc.tile_pool(name="inp", bufs=3) as inp,
            tc.tile_pool(name="outp", bufs=11) as outp,
        ):
            # ---- input-chunk loads: trace first so the sync engine's
            # instruction stream starts with them (nothing blocks them) ----
            its = []
            for k in range(NCH):
                it = inp.tile([P, CH], FP32)
                nc.sync.dma_start(it[:], ew[k])
                its.append(it)

            # ---- Filter output [C, R]: softmax over the free dim ----
            # weight values are tiny (std 0.01); plain exp is numerically
            # safe, matching jax.nn.softmax to fp32 rounding.
            w_sb = small.tile([C, R], FP32)
            nc.gpsimd.dma_start(w_sb[:], w[:])
            e_sb = small.tile([C, R], FP32)
            nc.scalar.activation(e_sb[:], w_sb[:],
                                 mybir.ActivationFunctionType.Exp)
            s4 = small.tile([C, 1], FP32)
            nc.vector.reduce_sum(s4[:], e_sb[:], axis=mybir.AxisListType.X)
            r4 = small.tile([C, 1], FP32)
            nc.vector.reciprocal(r4[:], s4[:])
            f_sb = small.tile([C, R], FP32)
            nc.vector.tensor_scalar_mul(f_sb[:], e_sb[:], r4[:, 0:1])
            nc.gpsimd.dma_start(filt[:], f_sb[:])

            # ---- fscal [P, C]: fscal[p, c] = Filter[c, p // JP] ----
            # Load weight transposed (one tiny DMA per channel), exp it,
            # then broadcast across partitions with two matmuls:
            #   rep[r, p]  = 1 iff p//JP == r   -> numerator per partition
            #   ones[r, p] = 1                  -> denominator per partition
            wT = small.tile([R, C], FP32)
            for c in range(C):
                nc.gpsimd.dma_start(wT[:, c:c + 1], w[c, :])
            eT = small.tile([R, C], FP32)
            nc.scalar.activation(eT[:], wT[:],
                                 mybir.ActivationFunctionType.Exp)
            rep = small.tile([R, P], FP32)
            nc.gpsimd.dma_start(rep[:], rep_dram[:])
            ones = small.tile([R, P], FP32)
            nc.vector.memset(ones[:], 1.0)
            ps_num = psum.tile([P, C], FP32)
            nc.tensor.matmul(ps_num[:], rep[:], eT[:])
            ps_den = psum.tile([P, C], FP32)
            nc.tensor.matmul(ps_den[:], ones[:], eT[:])
            rden = small.tile([P, C], FP32)
            nc.vector.reciprocal(rden[:], ps_den[:])
            fscal = small.tile([P, C], FP32)
            nc.vector.tensor_mul(fscal[:], ps_num[:], rden[:])

            # ---- main loop: scale each loaded chunk C times (DVE),
            # store from the scalar engine's HWDGE ring ----
            for k in range(NCH):
                it = its[k]
                for c in range(C):
                    ot = outp.tile([P, CH], FP32)
                    nc.vector.tensor_scalar_mul(ot[:], it[:],
                                                fscal[:, c:c + 1])
                    nc.scalar.dma_start(out[c, k], ot[:])

    nc.compile()
    return nc


def _get_nc():
    if "nc" not in _cached:
        _cached["nc"] = _build()
    return _cached["nc"]


def _shard_input(edge_w, k):
    """Core k's [R, ES] slice in device chunk-major layout [NCH, P, CH]."""
    shard = np.ascontiguousarray(
        edge_w[:, k * ES:(k + 1) * ES], dtype=np.float32)
    return np.ascontiguousarray(
        shard.reshape(P, NCH, CH).transpose(1, 0, 2))


def _unshard_output(dev_out):
    """Device [C, NCH, P, CH] chunk-major -> [C, R, ES]."""
    return dev_out.transpose(0, 2, 1, 3).reshape(C, R, ES)


def kernel(edge_w, weight):
    assert edge_w.shape == (R, E) and weight.shape == (C, R)
    nc = _get_nc()
    w_full = np.ascontiguousarray(weight, dtype=np.float32)
    in_maps = [{"ew": _shard_input(edge_w, k), "w": w_full}
               for k in range(NCORES)]
    res = run_bass_kernel_spmd(nc, in_maps, list(range(NCORES)))
    outs = [_unshard_output(res.results[k]["out"]) for k in range(NCORES)]
    full = np.concatenate(outs, axis=2)
    return full, res.results[0]["filt"]
